# revision 33
# baseline (speedup 1.0000x reference)
"""HeightAwarePointNetTiny on 8 Trainium2 NeuronCores (Bass/Tile).

Strategy: points of each cloud are z_eff-sorted on the host, so true KNN
neighbors lie within a sliding ~+-1km window of a query's sorted position.
Core (cloud b, half h) owns 4096 queries counted inward from "its" cloud
edge (h=1 bands are mirrored/descending), plus a 960-column inward halo, so
window offsets a(t) = clamp(128t + 64 - W/2, 0, NQ+960-W) are identical
compile-time constants on every core, and the cloud-edge clamp widens the
effective reach exactly like the reference full-scan does.

Per 128-query tile: PE emits the [128, 2048] window score matrix
(s = 2ci.cj - |ci|^2 - |cj|^2 + eps*rand_j, exact fp32; the tiny host-
provided random row breaks exact-tie duplicates in max_index) into PSUM;
Act stages an f32 copy to SBUF so PSUM frees early; DVE extracts the exact
top-16 per query with max8/max_index on PSUM then match_replace/max8/
max_index on the copy (5 passes).  Winner indices are stream-transposed
into the 16-partition wrapped layout and GPSIMD ap_gather fetches the
interleaved bf16 u-feature pairs channel-major.  The neighbor max-pool
runs as a log2 tree of bf16 tensor_tensor MAX ops (2x DVE mode), and the
i-side affine term v is accumulated in PSUM by PE (an identity matmul
adds the pooled max), so each block output is one Act relu+bias.

LocalAggBlock algebra (per point i): u_j = W_df f_j + W_dp p_j,
v_i = (W_f - W_df) f_i - W_dp p_i + b, out_i = relu(v_i + max_j u_j).
f1 halos: each core AllReduce-ADDs its reversed last-960 f1 columns and
subtracts its own contribution to recover the partner's (slot-free and
symmetric under the mirrored bands).  Global feature max-pool is a pair
AllReduce-max.  Weights are replicated; big matmuls run bf16.
"""
import sys, os
sys.path.insert(0, '/opt/trn_rl_repo')
import numpy as np
from contextlib import ExitStack

import concourse.bass as bass
import concourse.tile as tile
from concourse import bacc, mybir

dt = mybir.dt
F32 = dt.float32
F32R = dt.float32r
BF16 = dt.bfloat16
U16 = dt.uint16
I16 = dt.int16

B, N, IN_CH = 4, 8192, 4
K = 16
W0, W1, W2 = 64, 128, 256
NUM_CLASSES = 3
NCORES = 8
P = 128
CH = 512
NQ = N // 2                              # queries per core
NT = NQ // P                             # query tiles per core

WWIN = int(os.environ.get("WWIN", "2048"))   # candidate window per tile
HALO = 960                                   # inward halo width
NB = ((NQ + HALO + CH - 1) // CH) * CH       # band length (chunk-padded)
NCH = NB // CH
A_HI = NQ + HALO - WWIN                      # window-start clamp
BIGNEG = -3.0e38


def build_program(ncores=NCORES):
    nc = bacc.Bacc("TRN2", target_bir_lowering=False, debug=False,
                   num_devices=ncores)

    xT1 = nc.dram_tensor("xT1", [6, NB], F32, kind="ExternalInput")
    wm = {}
    for name, shape in [
        ("m_coords", [5, 3]), ("stem_w", [4, W0]), ("stem_b", [W0, 1]),
        ("w1_u_a", [W0, W1]), ("w1_u_b", [3, W1]),
        ("w1_v_a", [W0, W1]), ("w1_v_b", [3, W1]), ("b1_b", [W1, 1]),
        ("w2_u_a", [W1, W2]), ("w2_u_b", [3, W2]),
        ("w2_v_a", [W1, W2]), ("w2_v_b", [3, W2]), ("b2_b", [P, 2]),
        ("glob_k0", [P, W2]), ("glob_k1", [P, W2]), ("glob_b", [P, 2]),
        ("h1a_k0", [P, W2]), ("h1a_k1", [P, W2]),
        ("h1g_k0", [P, W2]), ("h1g_k1", [P, W2]), ("h1_b", [P, 2]),
        ("h2_k0", [P, NUM_CLASSES]), ("h2_k1", [P, NUM_CLASSES]),
        ("h2_b", [NUM_CLASSES, 1]), ("sig_par", [1, 3]),
        ("ident", [P, P]),
    ]:
        wm[name] = nc.dram_tensor(name, shape, F32, kind="ExternalInput")

    out_lg = nc.dram_tensor("out_lg", [NUM_CLASSES, NQ], F32,
                            kind="ExternalOutput")
    f1h_loc = nc.dram_tensor("f1h_loc", [P, HALO], F32)
    f1h_red = nc.dram_tensor("f1h_red", [P, HALO], F32)
    g_loc = nc.dram_tensor("g_loc", [W2, 1], F32)
    g_red = nc.dram_tensor("g_red", [W2, 1], F32)
    PAIRS = [[c, c + 1] for c in range(0, ncores, 2)] if ncores > 1 else []

    Relu = mybir.ActivationFunctionType.Relu
    Copy = mybir.ActivationFunctionType.Copy
    Sigmoid = mybir.ActivationFunctionType.Sigmoid
    Square = mybir.ActivationFunctionType.Square
    AX = mybir.AxisListType.X
    MAX = mybir.AluOpType.max
    ADD = mybir.AluOpType.add
    MULT = mybir.AluOpType.mult

    def r32(ap):
        return ap.bitcast(F32R)

    with tile.TileContext(nc) as tc, ExitStack() as ctx:
        pers = ctx.enter_context(tc.tile_pool(name="pers", bufs=1))
        lp = ctx.enter_context(tc.tile_pool(name="lp", bufs=2))
        ppS = ctx.enter_context(tc.tile_pool(name="ppS", bufs=1, space="PSUM"))
        ppM = ctx.enter_context(tc.tile_pool(name="ppM", bufs=2, space="PSUM"))
        ppV = ctx.enter_context(tc.tile_pool(name="ppV", bufs=2, space="PSUM"))

        # ---- load weights; convert matmul weights to bf16 ----
        W = {}
        for name in wm:
            t = pers.tile(list(wm[name].shape), F32, tag=name, name=name)
            nc.sync.dma_start(t[:], wm[name].ap())
            W[name] = t
        WB = {}
        for name in ["w1_u_a", "w1_u_b", "w1_v_a", "w1_v_b",
                     "w2_u_a", "w2_u_b", "w2_v_a", "w2_v_b",
                     "h1a_k0", "h1a_k1", "h2_k0", "h2_k1", "ident"]:
            t = pers.tile(list(wm[name].shape), BF16, tag=name + "b",
                          name=name + "b")
            nc.scalar.activation(t[:], W[name][:], Copy)
            WB[name] = t
        IDB = WB["ident"]

        # persistent state (q6 rows: 0:3 coords, 3 = -|ci|^2, 4 = ones;
        # scores use s = 2ci.cj + (-|ci|^2)*1 + 1*(-|cj|^2))
        q6 = pers.tile([6, NQ], F32, tag="q6")
        cchb = pers.tile([3, NB], BF16, tag="cchb")
        f1B = pers.tile([P, NB], BF16, tag="f1B")
        f2T = [pers.tile([P, NQ], BF16, tag=f"f2T{o}", name=f"f2T{o}")
               for o in range(2)]
        wrap = pers.tile([P, NT * P], U16, tag="wrap")
        for po in range(32, P, 32):
            nc.vector.memset(wrap[po:po + 32, :], 0)
        nc.vector.memset(f1B[:, NQ + HALO:NB], 0)
        ones3 = pers.tile([3, 1], F32, tag="ones3")
        nc.vector.memset(ones3[:], 1.0)
        ones1 = pers.tile([1, CH], F32, tag="ones1")
        nc.vector.memset(ones1[:], 1.0)

        def tree_max(src_view, out_view):
            """src_view: [c, ..., s]; repeatedly halve the last (s) axis via
            bf16 tensor_tensor MAX (2x DVE mode); final level -> out_view."""
            cur = src_view
            s = cur.shape[-1]
            while s > 2:
                s //= 2
                mid = int(np.prod(cur.shape[1:-1]))
                buf = lp.tile([P, mid * s], BF16, tag=f"trm{mid}x{s}")
                if len(cur.shape) == 3:
                    bv = buf[:].rearrange("c (r s) -> c r s", s=s)
                else:
                    bv = buf[:].rearrange("c (o r s) -> c o r s",
                                          o=cur.shape[1], s=s)
                nc.vector.tensor_tensor(bv, cur[..., 0:s], cur[..., s:2 * s],
                                        op=MAX)
                cur = bv
            nc.vector.tensor_tensor(out_view, cur[..., 0:1], cur[..., 1:2],
                                    op=MAX)

        with tc.tile_pool(name="sc1", bufs=1) as sc1, \
             tc.tile_pool(name="stg", bufs=2) as stg, \
             tc.tile_pool(name="sel", bufs=2) as sel, \
             tc.tile_pool(name="g1p", bufs=3) as g1p:
            rhs5 = sc1.tile([6, NB], F32, tag="rhs5")
            scc = sc1.tile([P, WWIN], F32, tag="scc")
            f64B = sc1.tile([W0, NB], BF16, tag="f64B")
            U1P = sc1.tile([P, 2 * NB], BF16, tag="U1P")
            nc.vector.memset(U1P[:], 0)
            U1Pv = U1P[:].rearrange("c (x h) -> c h x", h=2)

            # ---- candidate band setup: coords, rhs5, stem, u1 ----
            # rhs5 rows: [cx, cy, cz, 1, -|cj|^2];
            # q6 rows:   [2ci_x, 2ci_y, 2ci_z, -|ci|^2, 1]
            for i in range(NCH):
                sl = bass.ts(i, CH)
                xch = stg.tile([6, CH], F32, tag="xch")
                nc.sync.dma_start(xch[:], xT1.ap()[:, sl])
                psc = ppM.tile([3, CH], F32, tag="mm", name="csps")
                nc.tensor.matmul(psc[:], W["m_coords"][:], xch[0:5, :],
                                 start=True, stop=True)
                nc.scalar.activation(rhs5[0:3, sl], psc[:], Copy, scale=1.0)
                nc.scalar.activation(cchb[:, sl], psc[:], Copy, scale=1.0)
                sqs = stg.tile([3, CH], F32, tag="sqs")
                nc.scalar.activation(sqs[:], rhs5[0:3, sl], Square)
                psx = ppM.tile([1, CH], F32, tag="mm", name="xxps")
                nc.tensor.matmul(psx[:], ones3[:], sqs[:],
                                 start=True, stop=True)
                xxs = stg.tile([1, CH], F32, tag="xxs")
                nc.scalar.activation(xxs[:], psx[:], Copy, scale=-1.0)
                nc.sync.dma_start(rhs5[4:5, sl], xxs[:])
                nc.sync.dma_start(rhs5[3:4, sl], ones1[0:1, :])
                nc.sync.dma_start(rhs5[5:6, sl], xch[5:6, :])
                psf = ppM.tile([W0, CH], F32, tag="mm", name="stps")
                nc.tensor.matmul(psf[:], W["stem_w"][:], xch[0:4, :],
                                 start=True, stop=True)
                nc.scalar.activation(f64B[:, sl], psf[:], Relu,
                                     bias=W["stem_b"][:, 0:1], scale=1.0)
                # u1 = W_df f + W_dp p  (channel-major, even slots of U1P)
                psu = ppM.tile([P, CH], F32, tag="mm", name="u1ps")
                nc.tensor.matmul(psu[:], WB["w1_u_a"][:], f64B[:, sl],
                                 start=True, stop=False)
                nc.tensor.matmul(psu[:], WB["w1_u_b"][:], cchb[:, sl],
                                 start=False, stop=True)
                nc.scalar.activation(
                    U1Pv[:, 0:1, sl],
                    psu[:].rearrange("c (o x) -> c o x", o=1), Copy)

            # ---- query-side q6 rows from rhs5 ----
            qsl = slice(0, NQ)
            nc.scalar.activation(q6[0:3, :], rhs5[0:3, qsl], Copy, scale=2.0)
            nc.sync.dma_start(q6[3:4, :], rhs5[4:5, qsl])
            for j in range(NQ // CH):
                nc.sync.dma_start(q6[4:5, bass.ts(j, CH)], ones1[0:1, :])
                nc.sync.dma_start(q6[5:6, bass.ts(j, CH)], ones1[0:1, :])

            # ---- loop 1: window scores -> top16 -> gather u1 -> f1 ----
            pend1 = []

            def finish1(t):
                g1 = pend1.pop(0)[1]
                g1v = g1[:].bitcast(BF16).rearrange(
                    "c (q s h) -> c q h s", s=K, h=2)[:, :, 0:1, :]
                m1 = lp.tile([P, P], BF16, tag="m1")
                m1v = m1[:].rearrange("c (q s) -> c q s", s=1)
                lv1 = lp.tile([P, P * 8], BF16, tag="l1b")
                lv1v = lv1[:].rearrange("c (q s) -> c q s", s=8)
                nc.vector.tensor_tensor(lv1v, g1v[:, :, 0, 0:8],
                                        g1v[:, :, 0, 8:16], op=MAX)
                tree_max(lv1v, m1v)
                qts = slice(128 * t, 128 * t + P)
                psv = ppV.tile([P, P], F32, tag="vps", name="v1ps")
                nc.tensor.matmul(psv[:], WB["w1_v_a"][:], f64B[:, qts],
                                 start=True, stop=False)
                nc.tensor.matmul(psv[:], WB["w1_v_b"][:], cchb[:, qts],
                                 start=False, stop=False)
                nc.tensor.matmul(psv[:], IDB[:], m1[:], start=False,
                                 stop=True)
                nc.scalar.activation(f1B[:, qts], psv[:], Relu,
                                     bias=W["b1_b"][:, 0:1], scale=1.0)

            for t in range(NT):
                tsl = bass.ts(t, P)
                a = min(max(128 * t + 64 - WWIN // 2, 0), A_HI)
                ps = ppS.tile([P, WWIN], F32, tag="sc", name="scps")
                for j in range(WWIN // CH):
                    nc.tensor.matmul(ps[:, bass.ts(j, CH)],
                                     q6[:, tsl],
                                     rhs5[:, a + CH * j:a + CH * j + CH],
                                     start=True, stop=True)
                w8a = sel.tile([P, 8], F32, tag="w8a")
                w8b = sel.tile([P, 8], F32, tag="w8b")
                gip = sel.tile([P, 32], U16, tag="gip")
                nc.scalar.activation(scc[:], ps[:], Copy, scale=1.0)
                nc.vector.max(w8a[:], ps[:])
                nc.vector.max_index(gip[:, 0:8], w8a[:], ps[:])
                nc.vector.match_replace(scc[:], w8a[:], scc[:], BIGNEG)
                nc.vector.max(w8b[:], scc[:])
                nc.vector.max_index(gip[:, 8:16], w8b[:], scc[:])
                if a > 0:
                    nc.vector.tensor_scalar(gip[:, 0:16], gip[:, 0:16],
                                            float(a), None, op0=ADD)
                nc.vector.tensor_copy(gip[:, 16:32], gip[:, 0:16])
                for b_ in range(4):
                    nc.vector.transpose(
                        wrap[0:32, 128 * t + 32 * b_:128 * t + 32 * b_ + 32],
                        gip[32 * b_:32 * b_ + 32, 0:32])
                # rows 0:32 now hold [idx; idx]; replicate to rows 32:128
                for g_ in range(1, 4):
                    nc.sync.dma_start(wrap[32 * g_:32 * g_ + 32, tsl],
                                      wrap[0:32, tsl])
                g1 = g1p.tile([P, P * K], F32, tag="g1")
                nc.gpsimd.ap_gather(
                    g1[:].rearrange("c (n d) -> c n d", d=1),
                    U1P[:].bitcast(F32).rearrange("c (n d) -> c n d", d=1),
                    wrap[:, tsl].bitcast(I16),
                    channels=P, num_elems=NB, d=1, num_idxs=P * K)
                pend1.append((t, g1))
                if len(pend1) > 3:
                    finish1(pend1[0][0])
            while pend1:
                finish1(pend1[0][0])

        # ---- halo exchange of f1 within the pair ----
        # Both cores need the PARTNER's last-HALO f1 columns in reversed
        # order (bands are mirrored).  AllReduce-ADD of each core's own
        # reversed tail, then subtract own to recover the partner's.
        with tc.tile_pool(name="hx", bufs=1) as hx:
            own = hx.tile([P, HALO], F32, tag="own")
            tail = f1B[:, NQ - HALO:NQ]
            rev = bass.AP(tail.tensor, tail.offset + HALO - 1,
                          [list(tail.ap[0]), [-1, HALO]])
            nc.scalar.activation(own[:], rev, Copy, scale=1.0)
            nc.sync.dma_start(f1h_loc.ap(), own[:])
            if PAIRS:
                nc.gpsimd.collective_compute(
                    "AllReduce", ADD, replica_groups=PAIRS,
                    ins=[f1h_loc.ap()], outs=[f1h_red.ap()])
            else:
                nc.sync.dma_start(f1h_red.ap(), f1h_loc.ap())
            red = hx.tile([P, HALO], F32, tag="red")
            nc.sync.dma_start(red[:], f1h_red.ap())
            nc.vector.tensor_tensor(f1B[:, NQ:NQ + HALO], red[:], own[:],
                                    op=mybir.AluOpType.subtract)

        with tc.tile_pool(name="sc2", bufs=1) as sc2, \
             tc.tile_pool(name="stg2", bufs=2) as stg2, \
             tc.tile_pool(name="g2p", bufs=3) as g2p:
            hagq = sc2.tile([1, NQ], F32, tag="hagq")
            nc.sync.dma_start(hagq[:], xT1.ap()[3:4, 0:NQ])
            U2P = sc2.tile([P, 2 * NB], BF16, tag="U2P")
            U2Pv = U2P[:].rearrange("c (x h) -> c h x", h=2)
            # ---- u2 chunks (channel-major, halves interleaved per point) ----
            for i in range(NCH):
                sl = bass.ts(i, CH)
                for o in range(2):
                    osl = slice(P * o, P * o + P)
                    psu = ppM.tile([P, CH], F32, tag="mm", name="u2ps")
                    nc.tensor.matmul(psu[:], WB["w2_u_a"][:, osl],
                                     f1B[:, sl], start=True, stop=False)
                    nc.tensor.matmul(psu[:], WB["w2_u_b"][:, osl],
                                     cchb[:, sl], start=False, stop=True)
                    nc.scalar.activation(
                        U2Pv[:, o:o + 1, sl],
                        psu[:].rearrange("c (o x) -> c o x", o=1), Copy)

            # ---- loop 2: gather u2 -> f2 ----
            pend2 = []

            def finish2(t):
                g2 = pend2.pop(0)[1]
                g2v = g2[:].bitcast(BF16).rearrange(
                    "c (q s h) -> c q h s", s=K, h=2)
                m2 = lp.tile([P, 2 * P], BF16, tag="m2")
                m2v = m2[:].rearrange("c (o q s) -> c q o s", o=2, s=1)
                lv1 = lp.tile([P, P * 2 * 8], BF16, tag="l2b")
                lv1v = lv1[:].rearrange("c (q h s) -> c q h s", h=2, s=8)
                nc.vector.tensor_tensor(lv1v, g2v[:, :, :, 0:8],
                                        g2v[:, :, :, 8:16], op=MAX)
                tree_max(lv1v, m2v)
                qts = slice(128 * t, 128 * t + P)
                for o in range(2):
                    osl = slice(P * o, P * o + P)
                    psv = ppV.tile([P, P], F32, tag="vps", name="v2ps")
                    nc.tensor.matmul(psv[:], WB["w2_v_a"][:, osl],
                                     f1B[:, qts], start=True, stop=False)
                    nc.tensor.matmul(psv[:], WB["w2_v_b"][:, osl],
                                     cchb[:, qts], start=False, stop=False)
                    nc.tensor.matmul(psv[:], IDB[:], m2[:, osl],
                                     start=False, stop=True)
                    nc.scalar.activation(f2T[o][:, bass.ts(t, P)], psv[:],
                                         Relu, bias=W["b2_b"][:, o:o + 1],
                                         scale=1.0)

            for t in range(NT):
                tsl = bass.ts(t, P)
                g2 = g2p.tile([P, P * K], F32, tag="g2")
                nc.gpsimd.ap_gather(
                    g2[:].rearrange("c (n d) -> c n d", d=1),
                    U2P[:].bitcast(F32).rearrange("c (n d) -> c n d", d=1),
                    wrap[:, tsl].bitcast(I16),
                    channels=P, num_elems=NB, d=1, num_idxs=P * K)
                pend2.append((t, g2))
                if len(pend2) > 3:
                    finish2(pend2[0][0])
            while pend2:
                finish2(pend2[0][0])

            # ---- global max pool + glob MLP + b_eff ----
            gmx = pers.tile([P, 2], F32, tag="gmx")
            for o in range(2):
                nc.vector.tensor_reduce(gmx[:, o:o + 1], f2T[o][:],
                                        axis=AX, op=MAX)
                nc.sync.dma_start(g_loc.ap()[o * P:o * P + P, :],
                                  gmx[:, o:o + 1])
            if PAIRS:
                nc.gpsimd.collective_compute(
                    "AllReduce", MAX, replica_groups=PAIRS,
                    ins=[g_loc.ap()], outs=[g_red.ap()])
            else:
                nc.sync.dma_start(g_red.ap(), g_loc.ap())
            gsb = pers.tile([P, 2], F32, tag="gsb")
            nc.sync.dma_start(
                gsb[:], g_red.ap().rearrange("(r p) c -> p (r c)", r=2))
            g2t = pers.tile([P, 2], F32, tag="g2t")
            beff = pers.tile([P, 2], F32, tag="beff")
            for o in range(2):
                osl = slice(P * o, P * o + P)
                psg = ppV.tile([P, 1], F32, tag="vps", name="gps")
                nc.tensor.matmul(psg[:], W["glob_k0"][:, osl], gsb[:, 0:1],
                                 start=True, stop=False)
                nc.tensor.matmul(psg[:], W["glob_k1"][:, osl], gsb[:, 1:2],
                                 start=False, stop=True)
                nc.scalar.activation(g2t[:, o:o + 1], psg[:], Relu,
                                     bias=W["glob_b"][:, o:o + 1], scale=1.0)
            for o in range(2):
                osl = slice(P * o, P * o + P)
                psb = ppV.tile([P, 1], F32, tag="vps", name="bps")
                nc.tensor.matmul(psb[:], W["h1g_k0"][:, osl], g2t[:, 0:1],
                                 start=True, stop=False)
                nc.tensor.matmul(psb[:], W["h1g_k1"][:, osl], g2t[:, 1:2],
                                 start=False, stop=True)
                nc.vector.tensor_scalar(beff[:, o:o + 1], psb[:],
                                        W["h1_b"][:, o:o + 1], None, op0=ADD)

            # ---- loop 3: head ----
            for t in range(NT):
                tsl = bass.ts(t, P)
                hT = [lp.tile([P, P], BF16, tag=f"hT{o}", name=f"hT{o}")
                      for o in range(2)]
                for o in range(2):
                    osl = slice(P * o, P * o + P)
                    psh = ppV.tile([P, P], F32, tag="vps", name="hps")
                    nc.tensor.matmul(psh[:], WB["h1a_k0"][:, osl],
                                     f2T[0][:, tsl], start=True, stop=False)
                    nc.tensor.matmul(psh[:], WB["h1a_k1"][:, osl],
                                     f2T[1][:, tsl], start=False, stop=True)
                    nc.scalar.activation(hT[o][:], psh[:], Relu,
                                         bias=beff[:, o:o + 1], scale=1.0)
                ps3 = ppM.tile([NUM_CLASSES, P], F32, tag="mm", name="lps")
                nc.tensor.matmul(ps3[:], WB["h2_k0"][:], hT[0][:],
                                 start=True, stop=False)
                nc.tensor.matmul(ps3[:], WB["h2_k1"][:], hT[1][:],
                                 start=False, stop=True)
                lg = lp.tile([NUM_CLASSES, P], F32, tag="lg")
                nc.vector.tensor_scalar(lg[:], ps3[:], W["h2_b"][:, 0:1],
                                        None, op0=ADD)
                sg = lp.tile([1, P], F32, tag="sg")
                nc.scalar.activation(sg[:], hagq[0:1, tsl], Sigmoid,
                                     bias=W["sig_par"][0:1, 1:2],
                                     scale=W["sig_par"][0:1, 0:1])
                nc.vector.scalar_tensor_tensor(
                    lg[0:1, :], sg[:], W["sig_par"][0:1, 2:3],
                    lg[0:1, :], op0=MULT, op1=ADD)
                nc.sync.dma_start(out_lg.ap()[:, tsl], lg[:])

    nc.compile()
    return nc


def prep_inputs(x, hmix_a, hmix_b, hmix_c, stem_w, stem_b, b1_w, b1_b,
                b2_w, b2_b, glob_w, glob_b, head1_w, head1_b,
                head2_w, head2_b, thresh, sharp, scale, ncores=NCORES):
    """Host-side layout prep: z_eff sort per cloud + per-core band slices
    (data movement) and weight repacking."""
    f = np.float32
    x = np.asarray(x, f)
    ha, hb, hc = float(hmix_a), float(hmix_b), float(hmix_c)

    m_coords = np.zeros((5, 3), f)
    m_coords[0, 0] = 1.0
    m_coords[1, 1] = 1.0
    m_coords[2, 2] = ha
    m_coords[3, 2] = hb
    m_coords[4, 2] = hc

    b1_w = np.asarray(b1_w, f); b2_w = np.asarray(b2_w, f)
    w1_f, w1_df, w1_dp = b1_w[0:W0], b1_w[W0:2 * W0], b1_w[2 * W0:]
    w2_f, w2_df, w2_dp = b2_w[0:W1], b2_w[W1:2 * W1], b2_w[2 * W1:]
    head1_w = np.asarray(head1_w, f)
    glob_w = np.asarray(glob_w, f); head2_w = np.asarray(head2_w, f)

    com = {
        "m_coords": m_coords,
        "stem_w": np.asarray(stem_w, f),
        "stem_b": np.asarray(stem_b, f).reshape(W0, 1),
        "w1_u_a": np.ascontiguousarray(w1_df),
        "w1_u_b": np.ascontiguousarray(w1_dp),
        "w1_v_a": np.ascontiguousarray(w1_f - w1_df),
        "w1_v_b": np.ascontiguousarray(-w1_dp),
        "b1_b": np.asarray(b1_b, f).reshape(W1, 1),
        "w2_u_a": np.ascontiguousarray(w2_df),
        "w2_u_b": np.ascontiguousarray(w2_dp),
        "w2_v_a": np.ascontiguousarray(w2_f - w2_df),
        "w2_v_b": np.ascontiguousarray(-w2_dp),
        "b2_b": np.ascontiguousarray(np.asarray(b2_b, f).reshape(2, P).T),
        "glob_k0": np.ascontiguousarray(glob_w[0:P]),
        "glob_k1": np.ascontiguousarray(glob_w[P:2 * P]),
        "glob_b": np.ascontiguousarray(np.asarray(glob_b, f).reshape(2, P).T),
        "h1a_k0": np.ascontiguousarray(head1_w[0:P]),
        "h1a_k1": np.ascontiguousarray(head1_w[P:2 * P]),
        "h1g_k0": np.ascontiguousarray(head1_w[2 * P:3 * P]),
        "h1g_k1": np.ascontiguousarray(head1_w[3 * P:4 * P]),
        "h1_b": np.ascontiguousarray(np.asarray(head1_b, f).reshape(2, P).T),
        "h2_k0": np.ascontiguousarray(head2_w[0:P]),
        "h2_k1": np.ascontiguousarray(head2_w[P:2 * P]),
        "h2_b": np.asarray(head2_b, f).reshape(NUM_CLASSES, 1),
        "sig_par": np.array([[-float(sharp), float(sharp) * float(thresh),
                              float(scale)]], f),
        "ident": np.eye(P, dtype=f),
    }
    in_maps = []
    orders = []
    rng = np.random.default_rng(12345)
    tie = (1e-4 * (0.5 + 0.5 * rng.random(NB))).astype(f)[None]
    for b in range(B):
        zeff = ha * x[b, :, 2] + hb * x[b, :, 3] + hc
        order = np.argsort(zeff, kind="stable")
        orders.append(order)
        xs = x[b][order]                         # [N, 4] sorted
        for h in range(2):
            band = xs[0:NB] if h == 0 else xs[::-1][0:NB]
            xT1 = np.concatenate([band.T, np.ones((1, NB), f), tie], 0)
            in_maps.append({"xT1": np.ascontiguousarray(xT1), **com})
    return in_maps, orders


_CACHE = {}


def kernel(**inputs):
    from concourse.bass_utils import run_bass_kernel_spmd
    if "nc" not in _CACHE:
        _CACHE["nc"] = build_program()
    nc = _CACHE["nc"]
    in_maps, orders = prep_inputs(**inputs)
    r = run_bass_kernel_spmd(nc, in_maps, list(range(NCORES)))
    out = np.zeros((B, N, NUM_CLASSES), np.float32)
    for c in range(NCORES):
        b, h = c // 2, c % 2
        seg = r.results[c]["out_lg"].T           # [NQ, 3] band order
        if h == 0:
            out[b][orders[b][0:NQ]] = seg
        else:
            out[b][orders[b][NQ:2 * NQ]] = seg[::-1]
    return out


# revision 34
# speedup vs baseline: 1.2693x; 1.2693x over previous
"""HeightAwarePointNetTiny on 8 Trainium2 NeuronCores (Bass/Tile).

Strategy: points of each cloud are z_eff-sorted on the host, so true KNN
neighbors lie within a sliding ~+-1km window of a query's sorted position.
Core (cloud b, half h) owns 4096 queries counted inward from "its" cloud
edge (h=1 bands are mirrored/descending), plus a 960-column inward halo, so
window offsets a(t) = clamp(128t + 64 - W/2, 0, NQ+960-W) are identical
compile-time constants on every core, and the cloud-edge clamp widens the
effective reach exactly like the reference full-scan does.

Per 128-query tile: PE emits the [128, 2048] window score matrix
(s = 2ci.cj - |ci|^2 - |cj|^2 + eps*rand_j, exact fp32; the tiny host-
provided random row breaks exact-tie duplicates in max_index) into PSUM;
Act stages an f32 copy to SBUF so PSUM frees early; DVE extracts the exact
top-16 per query with max8/max_index on PSUM then match_replace/max8/
max_index on the copy (5 passes).  Winner indices are stream-transposed
into the 16-partition wrapped layout and GPSIMD ap_gather fetches the
interleaved bf16 u-feature pairs channel-major.  The neighbor max-pool
runs as a log2 tree of bf16 tensor_tensor MAX ops (2x DVE mode), and the
i-side affine term v is accumulated in PSUM by PE (an identity matmul
adds the pooled max), so each block output is one Act relu+bias.

LocalAggBlock algebra (per point i): u_j = W_df f_j + W_dp p_j,
v_i = (W_f - W_df) f_i - W_dp p_i + b, out_i = relu(v_i + max_j u_j).
f1 halos: each core AllReduce-ADDs its reversed last-960 f1 columns and
subtracts its own contribution to recover the partner's (slot-free and
symmetric under the mirrored bands).  Global feature max-pool is a pair
AllReduce-max.  Weights are replicated; big matmuls run bf16.
"""
import sys, os
sys.path.insert(0, '/opt/trn_rl_repo')
import numpy as np
from contextlib import ExitStack

import concourse.bass as bass
import concourse.tile as tile
from concourse import bacc, mybir

dt = mybir.dt
F32 = dt.float32
F32R = dt.float32r
BF16 = dt.bfloat16
U16 = dt.uint16
I16 = dt.int16

B, N, IN_CH = 4, 8192, 4
K = 16
W0, W1, W2 = 64, 128, 256
NUM_CLASSES = 3
NCORES = 8
P = 128
CH = 512
NQ = N // 2                              # queries per core
NT = NQ // P                             # query tiles per core

WWIN = int(os.environ.get("WWIN", "2048"))   # candidate window per tile
HALO = 960                                   # inward halo width
NB = ((NQ + HALO + CH - 1) // CH) * CH       # band length (chunk-padded)
NCH = NB // CH
A_HI = NQ + HALO - WWIN                      # window-start clamp
BIGNEG = -3.0e38


def build_program(ncores=NCORES):
    nc = bacc.Bacc("TRN2", target_bir_lowering=False, debug=False,
                   num_devices=ncores)

    xT1 = nc.dram_tensor("xT1", [6, NB], F32, kind="ExternalInput")
    wm = {}
    for name, shape in [
        ("m_coords", [5, 3]), ("stem_w", [4, W0]), ("stem_b", [W0, 1]),
        ("w1_u_a", [W0, W1]), ("w1_u_b", [3, W1]),
        ("w1_v_a", [W0, W1]), ("w1_v_b", [3, W1]), ("b1_b", [W1, 1]),
        ("w2_u_a", [W1, W2]), ("w2_u_b", [3, W2]),
        ("w2_v_a", [W1, W2]), ("w2_v_b", [3, W2]), ("b2_b", [P, 2]),
        ("glob_k0", [P, W2]), ("glob_k1", [P, W2]), ("glob_b", [P, 2]),
        ("h1a_k0", [P, W2]), ("h1a_k1", [P, W2]),
        ("h1g_k0", [P, W2]), ("h1g_k1", [P, W2]), ("h1_b", [P, 2]),
        ("h2_k0", [P, NUM_CLASSES]), ("h2_k1", [P, NUM_CLASSES]),
        ("h2_b", [NUM_CLASSES, 1]), ("sig_par", [1, 3]),
        ("ident", [P, P]),
    ]:
        wm[name] = nc.dram_tensor(name, shape, F32, kind="ExternalInput")

    out_lg = nc.dram_tensor("out_lg", [NUM_CLASSES, NQ], F32,
                            kind="ExternalOutput")
    f1h_loc = nc.dram_tensor("f1h_loc", [P, HALO], F32)
    f1h_red = nc.dram_tensor("f1h_red", [P, HALO], F32)
    g_loc = nc.dram_tensor("g_loc", [W2, 1], F32)
    g_red = nc.dram_tensor("g_red", [W2, 1], F32)
    PAIRS = [[c, c + 1] for c in range(0, ncores, 2)] if ncores > 1 else []

    Relu = mybir.ActivationFunctionType.Relu
    Copy = mybir.ActivationFunctionType.Copy
    Sigmoid = mybir.ActivationFunctionType.Sigmoid
    Square = mybir.ActivationFunctionType.Square
    AX = mybir.AxisListType.X
    MAX = mybir.AluOpType.max
    ADD = mybir.AluOpType.add
    MULT = mybir.AluOpType.mult

    def r32(ap):
        return ap.bitcast(F32R)

    with tile.TileContext(nc) as tc, ExitStack() as ctx:
        pers = ctx.enter_context(tc.tile_pool(name="pers", bufs=1))
        lp = ctx.enter_context(tc.tile_pool(name="lp", bufs=2))
        ppS = ctx.enter_context(tc.tile_pool(name="ppS", bufs=1, space="PSUM"))
        ppM = ctx.enter_context(tc.tile_pool(name="ppM", bufs=2, space="PSUM"))
        ppV = ctx.enter_context(tc.tile_pool(name="ppV", bufs=2, space="PSUM"))

        # ---- load weights; convert matmul weights to bf16 ----
        W = {}
        for name in wm:
            t = pers.tile(list(wm[name].shape), F32, tag=name, name=name)
            nc.sync.dma_start(t[:], wm[name].ap())
            W[name] = t
        WB = {}
        for name in ["w1_u_a", "w1_u_b", "w1_v_a", "w1_v_b",
                     "w2_u_a", "w2_u_b", "w2_v_a", "w2_v_b",
                     "h1a_k0", "h1a_k1", "h2_k0", "h2_k1", "ident"]:
            t = pers.tile(list(wm[name].shape), BF16, tag=name + "b",
                          name=name + "b")
            nc.scalar.activation(t[:], W[name][:], Copy)
            WB[name] = t
        IDB = WB["ident"]

        # persistent state (q6 rows: 0:3 coords, 3 = -|ci|^2, 4 = ones;
        # scores use s = 2ci.cj + (-|ci|^2)*1 + 1*(-|cj|^2))
        q6 = pers.tile([6, NQ], F32, tag="q6")
        cchb = pers.tile([3, NB], BF16, tag="cchb")
        f1B = pers.tile([P, NB], BF16, tag="f1B")
        f2T = [pers.tile([P, NQ], BF16, tag=f"f2T{o}", name=f"f2T{o}")
               for o in range(2)]
        wrap = pers.tile([P, NT * P], U16, tag="wrap")
        for po in range(32, P, 32):
            nc.vector.memset(wrap[po:po + 32, :], 0)
        nc.vector.memset(f1B[:, NQ + HALO:NB], 0)
        ones3 = pers.tile([3, 1], F32, tag="ones3")
        nc.vector.memset(ones3[:], 1.0)
        ones1 = pers.tile([1, CH], F32, tag="ones1")
        nc.vector.memset(ones1[:], 1.0)

        def tree_max(src_view, out_view):
            """src_view: [c, ..., s]; repeatedly halve the last (s) axis via
            bf16 tensor_tensor MAX (2x DVE mode); final level -> out_view."""
            cur = src_view
            s = cur.shape[-1]
            while s > 2:
                s //= 2
                mid = int(np.prod(cur.shape[1:-1]))
                buf = lp.tile([P, mid * s], BF16, tag=f"trm{mid}x{s}")
                if len(cur.shape) == 3:
                    bv = buf[:].rearrange("c (r s) -> c r s", s=s)
                else:
                    bv = buf[:].rearrange("c (o r s) -> c o r s",
                                          o=cur.shape[1], s=s)
                nc.vector.tensor_tensor(bv, cur[..., 0:s], cur[..., s:2 * s],
                                        op=MAX)
                cur = bv
            nc.vector.tensor_tensor(out_view, cur[..., 0:1], cur[..., 1:2],
                                    op=MAX)

        with tc.tile_pool(name="sc1", bufs=1) as sc1, \
             tc.tile_pool(name="stg", bufs=2) as stg, \
             tc.tile_pool(name="sel", bufs=2) as sel, \
             tc.tile_pool(name="g1p", bufs=3) as g1p:
            rhs5 = sc1.tile([6, NB], F32, tag="rhs5")
            scc = sc1.tile([P, WWIN], F32, tag="scc")
            f64B = sc1.tile([W0, NB], BF16, tag="f64B")
            U1P = sc1.tile([P, 2 * NB], BF16, tag="U1P")
            nc.vector.memset(U1P[:], 0)
            U1Pv = U1P[:].rearrange("c (x h) -> c h x", h=2)

            # ---- candidate band setup: coords, rhs5, stem, u1 ----
            # rhs5 rows: [cx, cy, cz, 1, -|cj|^2];
            # q6 rows:   [2ci_x, 2ci_y, 2ci_z, -|ci|^2, 1]
            for i in range(NCH):
                sl = bass.ts(i, CH)
                xch = stg.tile([6, CH], F32, tag="xch")
                nc.sync.dma_start(xch[:], xT1.ap()[:, sl])
                psc = ppM.tile([3, CH], F32, tag="mm", name="csps")
                nc.tensor.matmul(psc[:], W["m_coords"][:], xch[0:5, :],
                                 start=True, stop=True)
                nc.scalar.activation(rhs5[0:3, sl], psc[:], Copy, scale=1.0)
                nc.scalar.activation(cchb[:, sl], psc[:], Copy, scale=1.0)
                sqs = stg.tile([3, CH], F32, tag="sqs")
                nc.scalar.activation(sqs[:], rhs5[0:3, sl], Square)
                psx = ppM.tile([1, CH], F32, tag="mm", name="xxps")
                nc.tensor.matmul(psx[:], ones3[:], sqs[:],
                                 start=True, stop=True)
                xxs = stg.tile([1, CH], F32, tag="xxs")
                nc.scalar.activation(xxs[:], psx[:], Copy, scale=-1.0)
                nc.sync.dma_start(rhs5[4:5, sl], xxs[:])
                nc.sync.dma_start(rhs5[3:4, sl], ones1[0:1, :])
                nc.sync.dma_start(rhs5[5:6, sl], xch[5:6, :])
                psf = ppM.tile([W0, CH], F32, tag="mm", name="stps")
                nc.tensor.matmul(psf[:], W["stem_w"][:], xch[0:4, :],
                                 start=True, stop=True)
                nc.scalar.activation(f64B[:, sl], psf[:], Relu,
                                     bias=W["stem_b"][:, 0:1], scale=1.0)
                # u1 = W_df f + W_dp p  (channel-major, even slots of U1P)
                psu = ppM.tile([P, CH], F32, tag="mm", name="u1ps")
                nc.tensor.matmul(psu[:], WB["w1_u_a"][:], f64B[:, sl],
                                 start=True, stop=False)
                nc.tensor.matmul(psu[:], WB["w1_u_b"][:], cchb[:, sl],
                                 start=False, stop=True)
                nc.scalar.activation(
                    U1Pv[:, 0:1, sl],
                    psu[:].rearrange("c (o x) -> c o x", o=1), Copy)

            # ---- query-side q6 rows from rhs5 (per-chunk for overlap) ----
            for j in range(NQ // CH):
                js = bass.ts(j, CH)
                nc.scalar.activation(q6[0:3, js], rhs5[0:3, js], Copy,
                                     scale=2.0)
                nc.sync.dma_start(q6[3:4, js], rhs5[4:5, js])
                nc.sync.dma_start(q6[4:5, js], ones1[0:1, :])
                nc.sync.dma_start(q6[5:6, js], ones1[0:1, :])

            # ---- loop 1: window scores -> top16 -> gather u1 -> f1 ----
            pend1 = []

            def finish1(t):
                g1 = pend1.pop(0)[1]
                g1v = g1[:].bitcast(BF16).rearrange(
                    "c (q s h) -> c q h s", s=K, h=2)[:, :, 0:1, :]
                m1 = lp.tile([P, P], BF16, tag="m1")
                m1v = m1[:].rearrange("c (q s) -> c q s", s=1)
                lv1 = lp.tile([P, P * 8], BF16, tag="l1b")
                lv1v = lv1[:].rearrange("c (q s) -> c q s", s=8)
                nc.vector.tensor_tensor(lv1v, g1v[:, :, 0, 0:8],
                                        g1v[:, :, 0, 8:16], op=MAX)
                tree_max(lv1v, m1v)
                qts = slice(128 * t, 128 * t + P)
                psv = ppV.tile([P, P], F32, tag="vps", name="v1ps")
                nc.tensor.matmul(psv[:], WB["w1_v_a"][:], f64B[:, qts],
                                 start=True, stop=False)
                nc.tensor.matmul(psv[:], WB["w1_v_b"][:], cchb[:, qts],
                                 start=False, stop=False)
                nc.tensor.matmul(psv[:], IDB[:], m1[:], start=False,
                                 stop=True)
                nc.scalar.activation(f1B[:, qts], psv[:], Relu,
                                     bias=W["b1_b"][:, 0:1], scale=1.0)

            for t in range(NT):
                tsl = bass.ts(t, P)
                a = min(max(128 * t + 64 - WWIN // 2, 0), A_HI)
                ps = ppS.tile([P, WWIN], F32, tag="sc", name="scps")
                for j in range(WWIN // CH):
                    nc.tensor.matmul(ps[:, bass.ts(j, CH)],
                                     q6[:, tsl],
                                     rhs5[:, a + CH * j:a + CH * j + CH],
                                     start=True, stop=True)
                w8a = sel.tile([P, 8], F32, tag="w8a")
                w8b = sel.tile([P, 8], F32, tag="w8b")
                gip = sel.tile([P, 32], U16, tag="gip")
                nc.scalar.activation(scc[:], ps[:], Copy, scale=1.0)
                nc.vector.max(w8a[:], ps[:])
                nc.vector.max_index(gip[:, 0:8], w8a[:], ps[:])
                nc.vector.match_replace(scc[:], w8a[:], scc[:], BIGNEG)
                nc.vector.max(w8b[:], scc[:])
                nc.vector.max_index(gip[:, 8:16], w8b[:], scc[:])
                nc.vector.tensor_copy(gip[:, 16:32], gip[:, 0:16])
                for b_ in range(4):
                    nc.vector.transpose(
                        wrap[0:32, 128 * t + 32 * b_:128 * t + 32 * b_ + 32],
                        gip[32 * b_:32 * b_ + 32, 0:32])
                # rows 0:32 now hold [idx; idx]; replicate to rows 32:128
                for g_ in range(1, 4):
                    nc.sync.dma_start(wrap[32 * g_:32 * g_ + 32, tsl],
                                      wrap[0:32, tsl])
                g1 = g1p.tile([P, P * K], F32, tag="g1")
                nc.gpsimd.ap_gather(
                    g1[:].rearrange("c (n d) -> c n d", d=1),
                    U1P[:].bitcast(F32)[:, a:a + WWIN].rearrange(
                        "c (n d) -> c n d", d=1),
                    wrap[:, tsl].bitcast(I16),
                    channels=P, num_elems=WWIN, d=1, num_idxs=P * K)
                pend1.append((t, g1))
                if len(pend1) > 3:
                    finish1(pend1[0][0])
            while pend1:
                finish1(pend1[0][0])

        # ---- halo exchange of f1 within the pair ----
        # Both cores need the PARTNER's last-HALO f1 columns in reversed
        # order (bands are mirrored).  AllReduce-ADD of each core's own
        # reversed tail, then subtract own to recover the partner's.
        with tc.tile_pool(name="hx", bufs=1) as hx:
            own = hx.tile([P, HALO], F32, tag="own")
            tail = f1B[:, NQ - HALO:NQ]
            rev = bass.AP(tail.tensor, tail.offset + HALO - 1,
                          [list(tail.ap[0]), [-1, HALO]])
            nc.scalar.activation(own[:], rev, Copy, scale=1.0)
            nc.sync.dma_start(f1h_loc.ap(), own[:])
            if PAIRS:
                nc.gpsimd.collective_compute(
                    "AllReduce", ADD, replica_groups=PAIRS,
                    ins=[f1h_loc.ap()], outs=[f1h_red.ap()])
            else:
                nc.sync.dma_start(f1h_red.ap(), f1h_loc.ap())
            red = hx.tile([P, HALO], F32, tag="red")
            nc.sync.dma_start(red[:], f1h_red.ap())
            nc.vector.tensor_tensor(f1B[:, NQ:NQ + HALO], red[:], own[:],
                                    op=mybir.AluOpType.subtract)

        with tc.tile_pool(name="sc2", bufs=1) as sc2, \
             tc.tile_pool(name="stg2", bufs=2) as stg2, \
             tc.tile_pool(name="g2p", bufs=3) as g2p:
            hagq = sc2.tile([1, NQ], F32, tag="hagq")
            nc.sync.dma_start(hagq[:], xT1.ap()[3:4, 0:NQ])
            U2P = sc2.tile([P, 2 * NB], BF16, tag="U2P")
            U2Pv = U2P[:].rearrange("c (x h) -> c h x", h=2)
            # ---- u2 chunks (channel-major, halves interleaved per point) ----
            for i in range(NCH):
                sl = bass.ts(i, CH)
                for o in range(2):
                    osl = slice(P * o, P * o + P)
                    psu = ppM.tile([P, CH], F32, tag="mm", name="u2ps")
                    nc.tensor.matmul(psu[:], WB["w2_u_a"][:, osl],
                                     f1B[:, sl], start=True, stop=False)
                    nc.tensor.matmul(psu[:], WB["w2_u_b"][:, osl],
                                     cchb[:, sl], start=False, stop=True)
                    nc.scalar.activation(
                        U2Pv[:, o:o + 1, sl],
                        psu[:].rearrange("c (o x) -> c o x", o=1), Copy)

            # ---- loop 2: gather u2 -> f2 ----
            pend2 = []

            def finish2(t):
                g2 = pend2.pop(0)[1]
                g2v = g2[:].bitcast(BF16).rearrange(
                    "c (q s h) -> c q h s", s=K, h=2)
                m2 = lp.tile([P, 2 * P], BF16, tag="m2")
                m2v = m2[:].rearrange("c (o q s) -> c q o s", o=2, s=1)
                lv1 = lp.tile([P, P * 2 * 8], BF16, tag="l2b")
                lv1v = lv1[:].rearrange("c (q h s) -> c q h s", h=2, s=8)
                nc.vector.tensor_tensor(lv1v, g2v[:, :, :, 0:8],
                                        g2v[:, :, :, 8:16], op=MAX)
                tree_max(lv1v, m2v)
                qts = slice(128 * t, 128 * t + P)
                for o in range(2):
                    osl = slice(P * o, P * o + P)
                    psv = ppV.tile([P, P], F32, tag="vps", name="v2ps")
                    nc.tensor.matmul(psv[:], WB["w2_v_a"][:, osl],
                                     f1B[:, qts], start=True, stop=False)
                    nc.tensor.matmul(psv[:], WB["w2_v_b"][:, osl],
                                     cchb[:, qts], start=False, stop=False)
                    nc.tensor.matmul(psv[:], IDB[:], m2[:, osl],
                                     start=False, stop=True)
                    nc.scalar.activation(f2T[o][:, bass.ts(t, P)], psv[:],
                                         Relu, bias=W["b2_b"][:, o:o + 1],
                                         scale=1.0)

            for t in range(NT):
                tsl = bass.ts(t, P)
                a = min(max(128 * t + 64 - WWIN // 2, 0), A_HI)
                g2 = g2p.tile([P, P * K], F32, tag="g2")
                nc.gpsimd.ap_gather(
                    g2[:].rearrange("c (n d) -> c n d", d=1),
                    U2P[:].bitcast(F32)[:, a:a + WWIN].rearrange(
                        "c (n d) -> c n d", d=1),
                    wrap[:, tsl].bitcast(I16),
                    channels=P, num_elems=WWIN, d=1, num_idxs=P * K)
                pend2.append((t, g2))
                if len(pend2) > 3:
                    finish2(pend2[0][0])
            while pend2:
                finish2(pend2[0][0])

            # ---- global max pool + glob MLP + b_eff ----
            gmx = pers.tile([P, 2], F32, tag="gmx")
            for o in range(2):
                nc.vector.tensor_reduce(gmx[:, o:o + 1], f2T[o][:],
                                        axis=AX, op=MAX)
                nc.sync.dma_start(g_loc.ap()[o * P:o * P + P, :],
                                  gmx[:, o:o + 1])
            if PAIRS:
                nc.gpsimd.collective_compute(
                    "AllReduce", MAX, replica_groups=PAIRS,
                    ins=[g_loc.ap()], outs=[g_red.ap()])
            else:
                nc.sync.dma_start(g_red.ap(), g_loc.ap())
            gsb = pers.tile([P, 2], F32, tag="gsb")
            nc.sync.dma_start(
                gsb[:], g_red.ap().rearrange("(r p) c -> p (r c)", r=2))
            g2t = pers.tile([P, 2], F32, tag="g2t")
            beff = pers.tile([P, 2], F32, tag="beff")
            for o in range(2):
                osl = slice(P * o, P * o + P)
                psg = ppV.tile([P, 1], F32, tag="vps", name="gps")
                nc.tensor.matmul(psg[:], W["glob_k0"][:, osl], gsb[:, 0:1],
                                 start=True, stop=False)
                nc.tensor.matmul(psg[:], W["glob_k1"][:, osl], gsb[:, 1:2],
                                 start=False, stop=True)
                nc.scalar.activation(g2t[:, o:o + 1], psg[:], Relu,
                                     bias=W["glob_b"][:, o:o + 1], scale=1.0)
            for o in range(2):
                osl = slice(P * o, P * o + P)
                psb = ppV.tile([P, 1], F32, tag="vps", name="bps")
                nc.tensor.matmul(psb[:], W["h1g_k0"][:, osl], g2t[:, 0:1],
                                 start=True, stop=False)
                nc.tensor.matmul(psb[:], W["h1g_k1"][:, osl], g2t[:, 1:2],
                                 start=False, stop=True)
                nc.vector.tensor_scalar(beff[:, o:o + 1], psb[:],
                                        W["h1_b"][:, o:o + 1], None, op0=ADD)

            # ---- loop 3: head ----
            for t in range(NT):
                tsl = bass.ts(t, P)
                hT = [lp.tile([P, P], BF16, tag=f"hT{o}", name=f"hT{o}")
                      for o in range(2)]
                for o in range(2):
                    osl = slice(P * o, P * o + P)
                    psh = ppV.tile([P, P], F32, tag="vps", name="hps")
                    nc.tensor.matmul(psh[:], WB["h1a_k0"][:, osl],
                                     f2T[0][:, tsl], start=True, stop=False)
                    nc.tensor.matmul(psh[:], WB["h1a_k1"][:, osl],
                                     f2T[1][:, tsl], start=False, stop=True)
                    nc.scalar.activation(hT[o][:], psh[:], Relu,
                                         bias=beff[:, o:o + 1], scale=1.0)
                ps3 = ppM.tile([NUM_CLASSES, P], F32, tag="mm", name="lps")
                nc.tensor.matmul(ps3[:], WB["h2_k0"][:], hT[0][:],
                                 start=True, stop=False)
                nc.tensor.matmul(ps3[:], WB["h2_k1"][:], hT[1][:],
                                 start=False, stop=True)
                lg = lp.tile([NUM_CLASSES, P], F32, tag="lg")
                nc.vector.tensor_scalar(lg[:], ps3[:], W["h2_b"][:, 0:1],
                                        None, op0=ADD)
                sg = lp.tile([1, P], F32, tag="sg")
                nc.scalar.activation(sg[:], hagq[0:1, tsl], Sigmoid,
                                     bias=W["sig_par"][0:1, 1:2],
                                     scale=W["sig_par"][0:1, 0:1])
                nc.vector.scalar_tensor_tensor(
                    lg[0:1, :], sg[:], W["sig_par"][0:1, 2:3],
                    lg[0:1, :], op0=MULT, op1=ADD)
                nc.sync.dma_start(out_lg.ap()[:, tsl], lg[:])

    nc.compile()
    return nc


def prep_inputs(x, hmix_a, hmix_b, hmix_c, stem_w, stem_b, b1_w, b1_b,
                b2_w, b2_b, glob_w, glob_b, head1_w, head1_b,
                head2_w, head2_b, thresh, sharp, scale, ncores=NCORES):
    """Host-side layout prep: z_eff sort per cloud + per-core band slices
    (data movement) and weight repacking."""
    f = np.float32
    x = np.asarray(x, f)
    ha, hb, hc = float(hmix_a), float(hmix_b), float(hmix_c)

    m_coords = np.zeros((5, 3), f)
    m_coords[0, 0] = 1.0
    m_coords[1, 1] = 1.0
    m_coords[2, 2] = ha
    m_coords[3, 2] = hb
    m_coords[4, 2] = hc

    b1_w = np.asarray(b1_w, f); b2_w = np.asarray(b2_w, f)
    w1_f, w1_df, w1_dp = b1_w[0:W0], b1_w[W0:2 * W0], b1_w[2 * W0:]
    w2_f, w2_df, w2_dp = b2_w[0:W1], b2_w[W1:2 * W1], b2_w[2 * W1:]
    head1_w = np.asarray(head1_w, f)
    glob_w = np.asarray(glob_w, f); head2_w = np.asarray(head2_w, f)

    com = {
        "m_coords": m_coords,
        "stem_w": np.asarray(stem_w, f),
        "stem_b": np.asarray(stem_b, f).reshape(W0, 1),
        "w1_u_a": np.ascontiguousarray(w1_df),
        "w1_u_b": np.ascontiguousarray(w1_dp),
        "w1_v_a": np.ascontiguousarray(w1_f - w1_df),
        "w1_v_b": np.ascontiguousarray(-w1_dp),
        "b1_b": np.asarray(b1_b, f).reshape(W1, 1),
        "w2_u_a": np.ascontiguousarray(w2_df),
        "w2_u_b": np.ascontiguousarray(w2_dp),
        "w2_v_a": np.ascontiguousarray(w2_f - w2_df),
        "w2_v_b": np.ascontiguousarray(-w2_dp),
        "b2_b": np.ascontiguousarray(np.asarray(b2_b, f).reshape(2, P).T),
        "glob_k0": np.ascontiguousarray(glob_w[0:P]),
        "glob_k1": np.ascontiguousarray(glob_w[P:2 * P]),
        "glob_b": np.ascontiguousarray(np.asarray(glob_b, f).reshape(2, P).T),
        "h1a_k0": np.ascontiguousarray(head1_w[0:P]),
        "h1a_k1": np.ascontiguousarray(head1_w[P:2 * P]),
        "h1g_k0": np.ascontiguousarray(head1_w[2 * P:3 * P]),
        "h1g_k1": np.ascontiguousarray(head1_w[3 * P:4 * P]),
        "h1_b": np.ascontiguousarray(np.asarray(head1_b, f).reshape(2, P).T),
        "h2_k0": np.ascontiguousarray(head2_w[0:P]),
        "h2_k1": np.ascontiguousarray(head2_w[P:2 * P]),
        "h2_b": np.asarray(head2_b, f).reshape(NUM_CLASSES, 1),
        "sig_par": np.array([[-float(sharp), float(sharp) * float(thresh),
                              float(scale)]], f),
        "ident": np.eye(P, dtype=f),
    }
    in_maps = []
    orders = []
    rng = np.random.default_rng(12345)
    tie = (1e-4 * (0.5 + 0.5 * rng.random(NB))).astype(f)[None]
    for b in range(B):
        zeff = ha * x[b, :, 2] + hb * x[b, :, 3] + hc
        order = np.argsort(zeff, kind="stable")
        orders.append(order)
        xs = x[b][order]                         # [N, 4] sorted
        for h in range(2):
            band = xs[0:NB] if h == 0 else xs[::-1][0:NB]
            xT1 = np.concatenate([band.T, np.ones((1, NB), f), tie], 0)
            in_maps.append({"xT1": np.ascontiguousarray(xT1), **com})
    return in_maps, orders


_CACHE = {}


def kernel(**inputs):
    from concourse.bass_utils import run_bass_kernel_spmd
    if "nc" not in _CACHE:
        _CACHE["nc"] = build_program()
    nc = _CACHE["nc"]
    in_maps, orders = prep_inputs(**inputs)
    r = run_bass_kernel_spmd(nc, in_maps, list(range(NCORES)))
    out = np.zeros((B, N, NUM_CLASSES), np.float32)
    for c in range(NCORES):
        b, h = c // 2, c % 2
        seg = r.results[c]["out_lg"].T           # [NQ, 3] band order
        if h == 0:
            out[b][orders[b][0:NQ]] = seg
        else:
            out[b][orders[b][NQ:2 * NQ]] = seg[::-1]
    return out
